# revision 3
# baseline (speedup 1.0000x reference)
"""Sliding-window attention kernel for 8 Trainium2 NeuronCores (v2, bf16).

Model (per reference): RMSNorm -> fused QKV -> partial RoPE(32 dims) ->
sliding-window causal attention (window 1024) -> output projection.
Shapes: x [1, 4096, 2048], 16 heads x 128 dim, rope on first 32 dims.

Sharding: Megatron-style tensor parallel across heads. Each of the 8 cores
owns 2 heads, computes a dense partial output [4096, 2048] (bf16), and the
host sums the 8 partials in fp32.

Key device-side choices (vs the fp32r baseline):
- Everything bf16: halves LDWEIGHTS time (the baseline bottleneck: 554us of
  fp32 weight loads serializing on the tensor NX), doubles DVE throughput,
  halves DMA bytes.
- CHUNK=512 in the QKV phase: N=512 matmuls halve the LDWEIGHTS count.
- V is produced directly in natural [seq, dim] layout (lhsT = x^T tile,
  moving = Wv^T), killing the baseline's 64 PE transposes.
- RMSNorm r: Q^T and K^T get it folded into their RoPE cos/sin multiplies
  (cosrb = cos*r, free); V gets it as a per-partition tensor_scalar fused
  into its mandatory PSUM->SBUF copy, using r columns built from tiny PE
  transposes of the ms row.
- Exp runs on PAIRED score strips ([128,1024] per ACT op) and the phase-B
  emission zips score pairs of quad p with sum/AV/oproj matmuls of quad
  p-1, so the scalar engine's exp stream hides behind PE work.
- Q stays in SBUF for phase B (no DRAM roundtrip); out partials bf16.
"""

import sys

sys.path.insert(0, "/opt/trn_rl_repo")

import numpy as np

import concourse.bacc as bacc
import concourse.bass as bass
import concourse.tile as tile
from concourse import bass_utils, mybir

F32 = mybir.dt.float32
BF16 = mybir.dt.bfloat16
AF = mybir.ActivationFunctionType
OP = mybir.AluOpType

B, S, H = 1, 4096, 2048
NH, HD = 16, 128
ROPE_N = 32
WINDOW = 1024
EPS = 1e-5
NCORES = 8
HPC = NH // NCORES          # heads per core = 2
CHUNK = 512                 # seq chunk for the QKV phase
NCHUNK = S // CHUNK         # 8
QUAD = 512                  # queries per attention block
NQUAD = S // QUAD           # 8
NKT = S // 128              # 32 key tiles per head
HT = H // 128               # 16 h-tiles
EXP_SCALE = 1.0 / np.sqrt(HD)

_CACHED = {}


def _install_ntff_hook():
    """Register the axon NTFF profile hook (the boot-time install is
    skipped when antenv.axon_hooks is missing from the image)."""
    import contextlib
    import ctypes
    import types

    if "antenv.axon_hooks" not in sys.modules:
        mod = types.ModuleType("antenv.axon_hooks")
        mod._hook = None
        mod.set_axon_ntff_profile_hook = lambda h: setattr(mod, "_hook", h)
        mod.get_axon_ntff_profile_hook = lambda: mod._hook
        sys.modules["antenv.axon_hooks"] = mod
    mod = sys.modules["antenv.axon_hooks"]
    if mod.get_axon_ntff_profile_hook() is not None:
        return
    try:
        lib = ctypes.CDLL("/opt/axon/libaxon_pjrt.so")
        if not hasattr(lib, "axon_start_nrt_profile"):
            return
    except OSError:
        return
    lib.axon_start_nrt_profile.argtypes = [
        ctypes.POINTER(ctypes.c_int64), ctypes.c_size_t]
    lib.axon_start_nrt_profile.restype = ctypes.c_int64
    lib.axon_stop_nrt_profile.argtypes = [ctypes.c_char_p]
    lib.axon_stop_nrt_profile.restype = ctypes.c_int64

    @contextlib.contextmanager
    def _hook(output_dir, device_ids):
        import jax
        jax.devices()
        if device_ids:
            ids = (ctypes.c_int64 * len(device_ids))(*device_ids)
            rc = lib.axon_start_nrt_profile(ids, len(device_ids))
        else:
            rc = lib.axon_start_nrt_profile(None, 0)
        if rc != 0:
            raise RuntimeError(f"axon_start_nrt_profile rc={rc}")
        try:
            yield
        finally:
            n = lib.axon_stop_nrt_profile(str(output_dir).encode())
            print(f"ntff profile: {n} file(s) written to {output_dir}",
                  file=sys.stderr)

    mod.set_axon_ntff_profile_hook(_hook)


def _build_program():
    nc = bacc.Bacc("TRN2", target_bir_lowering=False, debug=False)

    xT_d = nc.dram_tensor("xT", [H, S], BF16, kind="ExternalInput")
    xn_d = nc.dram_tensor("xn", [S, H], BF16, kind="ExternalInput")
    identf_d = nc.dram_tensor("identf", [128, 128], F32, kind="ExternalInput")
    wqk_d = nc.dram_tensor("wqk", [4, 128, HT, 128], BF16, kind="ExternalInput")
    wv_d = nc.dram_tensor("wv", [128, HT, HPC * 128], BF16, kind="ExternalInput")
    ow_d = nc.dram_tensor("ow", [128, HPC, H], BF16, kind="ExternalInput")
    cos_d = nc.dram_tensor("cosext", [128, S], BF16, kind="ExternalInput")
    sin_d = nc.dram_tensor("sinext", [ROPE_N, S], BF16, kind="ExternalInput")
    smat_d = nc.dram_tensor("smat", [ROPE_N, ROPE_N], BF16, kind="ExternalInput")
    ones_d = nc.dram_tensor("ones", [128, 1], BF16, kind="ExternalInput")
    onef_d = nc.dram_tensor("onef", [1, 1], F32, kind="ExternalInput")
    tri_d = nc.dram_tensor("tri", [128, 2, 128], BF16, kind="ExternalInput")
    out_d = nc.dram_tensor("out", [S, H], BF16, kind="ExternalOutput")

    with tile.TileContext(nc) as tc:
        with nc.allow_low_precision(reason="bf16 kernel; rel tol 2e-2"):
            _emit(nc, tc, xT_d, xn_d, identf_d, wqk_d, wv_d, ow_d, cos_d,
                  sin_d, smat_d, ones_d, onef_d, tri_d, out_d)
    nc.compile()
    return nc


def _emit(nc, tc, xT_d, xn_d, identf_d, wqk_d, wv_d, ow_d, cos_d,
          sin_d, smat_d, ones_d, onef_d, tri_d, out_d):
    from contextlib import ExitStack

    xT_t = xT_d.ap().rearrange("(ho p) s -> p ho s", p=128)

    with ExitStack() as ctx:
        singles = ctx.enter_context(tc.tile_pool(name="singles", bufs=1))

        ow_sb = singles.tile([128, HPC, H], BF16)
        ones_sb = singles.tile([128, 1], BF16)
        nc.sync.dma_start(ones_sb[:], ones_d.ap())
        onef_sb = singles.tile([1, 1], F32)
        nc.sync.dma_start(onef_sb[:], onef_d.ap())
        identf_sb = singles.tile([128, 128], F32)
        nc.sync.dma_start(identf_sb[:], identf_d.ap())
        smat_sb = singles.tile([ROPE_N, ROPE_N], BF16)
        nc.sync.dma_start(smat_sb[:], smat_d.ap())
        eps_sb = singles.tile([1, 1], F32)
        nc.vector.memset(eps_sb[:], EPS)
        epsc_sb = singles.tile([128, 1], F32)
        nc.vector.memset(epsc_sb[:], EPS)

        # Resident K^T, V, Q^T for the attention phase
        kt_sb = singles.tile([128, HPC, S], BF16)           # [d, head, s]
        v_sb = singles.tile([128, HPC, NKT, 128], BF16)     # [s_in, head, s_tile, d]
        qt_sb = singles.tile([128, HPC, S], BF16)           # [d, head, s]

        # ---------------- Phase A: stats + QKV + RoPE ----------------
        with ExitStack() as actx:
            wpool = actx.enter_context(tc.tile_pool(name="wpool", bufs=1))
            wqk_sb = []
            for ot in range(4):
                wt = wpool.tile([128, HT, 128], BF16, name=f"wqk{ot}")
                wqk_sb.append(wt)
            wv_sb = wpool.tile([128, HT, HPC * 128], BF16)

            xpool = actx.enter_context(tc.tile_pool(name="xpool", bufs=2))
            dpool = actx.enter_context(tc.tile_pool(name="dpool", bufs=2))
            tspool = actx.enter_context(tc.tile_pool(name="tspool", bufs=2))

            qk_ps = actx.enter_context(
                tc.tile_pool(name="qk_ps", bufs=1, space="PSUM"))
            v_ps = actx.enter_context(
                tc.tile_pool(name="v_ps", bufs=1, space="PSUM"))
            aux_ps = actx.enter_context(
                tc.tile_pool(name="aux_ps", bufs=1, space="PSUM"))

            for c in range(NCHUNK):
                sl = slice(c * CHUNK, (c + 1) * CHUNK)
                xtg = []
                for g in range(4):
                    t = xpool.tile([128, 4, CHUNK], BF16, tag=f"xt{g}",
                                   name=f"xt{g}")
                    nc.sync.dma_start(t[:], xT_t[:, 4 * g:4 * (g + 1), sl])
                    xtg.append(t)
                if c == 0:
                    # weight loads interleave with chunk 0's x tiles; the
                    # phase-B-only ow load goes last
                    for ot in range(4):
                        nc.sync.dma_start(wqk_sb[ot][:], wqk_d.ap()[ot])
                    nc.sync.dma_start(wv_sb[:], wv_d.ap())
                xn = xpool.tile([128, 4, H], BF16, tag="xn")
                nc.sync.dma_start(
                    xn[:], xn_d.ap()[c * CHUNK:(c + 1) * CHUNK, :].rearrange(
                        "(st p) h -> p st h", p=128))
                cos_t = xpool.tile([128, CHUNK], BF16, tag="cos")
                nc.sync.dma_start(cos_t[:], cos_d.ap()[:, sl])
                sin_t = xpool.tile([ROPE_N, CHUNK], BF16, tag="sin")
                nc.sync.dma_start(sin_t[:], sin_d.ap()[:, sl])
                if c == 0:
                    nc.sync.dma_start(ow_sb[:], ow_d.ap())

                # sum(x^2) per seq position via ACT Square + accum_out on
                # natural-layout x (free-dim reduce on the idle scalar engine)
                mscol = dpool.tile([128, 4], F32, tag="mscol")
                sqscr = dpool.tile([128, H], BF16, tag="sqscr")
                for st in range(4):
                    nc.scalar.activation(
                        sqscr[:], xn[:, st, :], AF.Square,
                        accum_out=mscol[:, st:st + 1])

                # QK matmuls: o-tiles 0..3 = Q_h0, K_h0, Q_h1, K_h1 -> [d, s]
                # (N=256 matmuls pace ~15% better than N=512 on HW)
                qkps = {}
                for ot in range(4):
                    ps = qk_ps.tile([128, CHUNK], F32, tag=f"qk{ot}")
                    for ha in range(2):
                        for ht in range(HT):
                            nc.tensor.matmul(
                                ps[:, ha * 256:(ha + 1) * 256],
                                wqk_sb[ot][:, ht, :],
                                xtg[ht // 4][:, ht % 4,
                                             ha * 256:(ha + 1) * 256],
                                start=(ht == 0), stop=(ht == HT - 1))
                    qkps[ot] = ps
                    if ot == 2:
                        # r columns: sq = sqrt(ms/H + eps); rcol = 1/sq
                        sqcol = dpool.tile([128, 4], F32, tag="sqcol")
                        nc.scalar.activation(sqcol[:], mscol[:], AF.Sqrt,
                                             bias=epsc_sb[:], scale=1.0 / H)
                        rcol = dpool.tile([128, 4], F32, tag="rcol")
                        nc.vector.reciprocal_approx_fast(rcol[:], sqcol[:])
                        # r row: transpose the r columns into a [1,512] psum
                        # row, then broadcast to all partitions
                        rrps = aux_ps.tile([1, CHUNK], F32, tag="aux")
                        for st in range(4):
                            nc.tensor.transpose(
                                rrps[:, st * 128:(st + 1) * 128],
                                rcol[:, st:st + 1], identf_sb[:])
                        rrow = dpool.tile([1, CHUNK], BF16, tag="rrow")
                        nc.vector.tensor_copy(rrow[:], rrps[:])
                        rb = dpool.tile([128, CHUNK], BF16, tag="rb")
                        nc.gpsimd.partition_broadcast(rb[:], rrow[:])

                # V in natural [s, d] layout: lhsT = x^T tile, rhs = Wv^T
                vps = [v_ps.tile([128, 2, HPC * 128], F32, tag=f"v{i}",
                                 name=f"vps{i}")
                       for i in range(2)]
                for st in range(4):
                    ps = vps[st // 2]
                    for ht in range(HT):
                        nc.tensor.matmul(
                            ps[:, st % 2, :],
                            xtg[ht // 4][:, ht % 4,
                                         st * 128:(st + 1) * 128],
                            wv_sb[:, ht, :],
                            start=(ht == 0), stop=(ht == HT - 1))

                # cos*r and sin*r rows: fold the rmsnorm scale into RoPE
                cosrb = dpool.tile([128, CHUNK], BF16, tag="cosrb")
                nc.vector.tensor_tensor(cosrb[:], cos_t[:], rb[:], OP.mult)
                sinrb = dpool.tile([ROPE_N, CHUNK], BF16, tag="sinrb")
                nc.vector.tensor_tensor(
                    sinrb[:], sin_t[:], rb[:ROPE_N, :], OP.mult)

                # RoPE + rmsnorm scale for Q and K, writing qt_sb/kt_sb
                for ot in range(4):
                    head = ot // 2
                    is_k = ot % 2 == 1
                    ps = qkps[ot]
                    dst = (kt_sb if is_k else qt_sb)[:, head, sl]
                    tsin = tspool.tile([ROPE_N, CHUNK], BF16, tag=f"ts{ot}")
                    nc.vector.tensor_tensor(
                        tsin[:], ps[:ROPE_N, :], sinrb[:], OP.mult)
                    rope_ps = aux_ps.tile([ROPE_N, CHUNK], F32, tag="aux")
                    nc.tensor.matmul(rope_ps[:], smat_sb[:], tsin[:],
                                     start=True, stop=True)
                    nc.vector.tensor_tensor(dst, ps[:], cosrb[:], OP.mult)
                    nc.vector.tensor_tensor(
                        dst[:ROPE_N, :], dst[:ROPE_N, :], rope_ps[:], OP.add)

                # V psum -> SBUF with the r scale fused (per-partition)
                for st in range(4):
                    ps = vps[st // 2]
                    stile = c * 4 + st
                    for h in range(HPC):
                        nc.vector.tensor_scalar_mul(
                            v_sb[:, h, stile, :],
                            ps[:, st % 2, h * 128:(h + 1) * 128],
                            rcol[:, st:st + 1])

        # Preload the exp table set while the scalar engine is idle so the
        # first real exp doesn't eat the ~2.6us ACT_TABLE_LOAD.
        dummy_sb = singles.tile([1, 1], F32)
        nc.scalar.activation(dummy_sb[:], eps_sb[:], AF.Exp)

        # ---------------- Phase B: attention + output projection ----------------
        # Window-trimmed strips: strip t for quad p covers only the valid
        # query range (off, w); edge strips get a single [128,128] triangle
        # mask; sum/AV chains order a full-width strip first so the
        # has_written coverage is correct; all N=512 matmuls are split into
        # N=256 pairs (better HW issue pace).
        with ExitStack() as bctx:
            ppool = bctx.enter_context(tc.tile_pool(name="ppool", bufs=2))
            prpool = bctx.enter_context(tc.tile_pool(name="prpool", bufs=20))
            opool = bctx.enter_context(tc.tile_pool(name="opool", bufs=4))
            sc_ps = bctx.enter_context(
                tc.tile_pool(name="sc_ps", bufs=2, space="PSUM"))
            ao_ps = bctx.enter_context(
                tc.tile_pool(name="ao_ps", bufs=2, space="PSUM"))
            sm_ps = bctx.enter_context(
                tc.tile_pool(name="sm_ps", bufs=2, space="PSUM"))
            maskpool = bctx.enter_context(tc.tile_pool(name="maskpool", bufs=1))
            tri_sb = maskpool.tile([128, 2, 128], BF16)
            nc.sync.dma_start(tri_sb[:], tri_d.ap())

            def geom(t, p):
                """(off, w, mask) for strip t of quad p; mask = (col, which)
                for the [128,128] triangle block or None."""
                d = t - 4 * p
                if d >= 0:
                    off, w = 128 * d, 512 - 128 * d
                    return off, w, (off, 0)            # causalT at valid start
                if d >= -4:
                    return 0, 512, None
                w = 128 * (d + 9)
                return 0, w, (w - 128, 1)              # antiT at valid end

            plists = {}   # (p, h) -> list of (t, probs tile, off, w)

            def pair_list(p):
                tlist = list(range(max(0, 4 * p - 8), 4 * p + 4))
                return [(tlist[2 * j], tlist[2 * j + 1])
                        for j in range(len(tlist) // 2)]

            def emit_score_pair(p, h, t0, t1):
                """Two trimmed score strips -> exp -> triangle masks."""
                sc = sc_ps.tile([128, 2, QUAD], F32, tag="sc")
                geos = [geom(t, p) for t in (t0, t1)]
                for j, t in enumerate((t0, t1)):
                    off, w, _ = geos[j]
                    for o2 in range(off, off + w, 256):
                        w2 = min(256, off + w - o2)
                        nc.tensor.matmul(
                            sc[:, j, o2:o2 + w2],
                            kt_sb[:, h, t * 128:(t + 1) * 128],
                            qt_sb[:, h, p * QUAD + o2:p * QUAD + o2 + w2],
                            start=True, stop=True)
                probs = prpool.tile([128, 2, QUAD], BF16, tag="probs")
                if geos[0][1] + geos[1][1] >= 768:
                    # near-full pair: one big exp (garbage regions unread)
                    nc.scalar.activation(
                        probs[:].rearrange("p a b -> p (a b)"),
                        sc[:].rearrange("p a b -> p (a b)"),
                        AF.Exp, scale=float(EXP_SCALE))
                else:
                    for j in range(2):
                        off, w, _ = geos[j]
                        nc.scalar.activation(
                            probs[:, j, off:off + w], sc[:, j, off:off + w],
                            AF.Exp, scale=float(EXP_SCALE))
                for j in range(2):
                    off, w, mask = geos[j]
                    if mask is not None:
                        col, which = mask
                        nc.vector.tensor_tensor(
                            probs[:, j, col:col + 128],
                            probs[:, j, col:col + 128],
                            tri_sb[:, which, :], OP.mult)
                plists.setdefault((p, h), []).extend(
                    [(t0, probs, 0), (t1, probs, 1)])

            def strip_order(plist, p):
                """Full-width strip first (has_written coverage), then rest."""
                return sorted(
                    plist, key=lambda e: 0 if geom(e[0], p)[:2] == (0, 512)
                    else 1)

            def tail_groups(p):
                """Yield thunks emitting quad p's sum/AV/normalize/oproj in
                PE-contiguous groups, zipped between score pairs."""
                attn_sb = {}

                def chain_mms(plist, out_of, lhsT_of):
                    n = len(plist)
                    first = True
                    for ji, (t, pr, j) in enumerate(plist):
                        off, w, _ = geom(t, p)
                        for o2 in range(off, off + w, 256):
                            w2 = min(256, off + w - o2)
                            last = (ji == n - 1) and (o2 + 256 >= off + w)
                            nc.tensor.matmul(
                                out_of(o2, w2), lhsT_of(t),
                                pr[:, j, o2:o2 + w2],
                                start=first, stop=last)
                            first = False

                def head_sum(h):
                    plist = strip_order(plists[(p, h)], p)
                    sm = sm_ps.tile([1, QUAD], F32, tag="sm")
                    chain_mms(plist, lambda o2, w2: sm[:, o2:o2 + w2],
                              lambda t: ones_sb[:])
                    rrow = ppool.tile([1, QUAD], F32, tag="sums")
                    nc.vector.reciprocal_approx_fast(rrow[:], sm[:])
                    rrowb = ppool.tile([1, QUAD], BF16, tag="sumsb")
                    nc.vector.tensor_copy(rrowb[:], rrow[:])
                    recb = ppool.tile([128, QUAD], BF16, tag="recb")
                    nc.gpsimd.partition_broadcast(recb[:], rrowb[:])
                    attn_sb[h] = ("recb", recb)

                def head_av(h):
                    plist = strip_order(plists.pop((p, h)), p)
                    at = ao_ps.tile([128, QUAD], F32, tag="ao")
                    chain_mms(plist, lambda o2, w2: at[:, o2:o2 + w2],
                              lambda t: v_sb[:, h, t, :])
                    recb = attn_sb[h][1]
                    asb = ppool.tile([128, QUAD], BF16, tag=f"attn{h}")
                    nc.vector.tensor_tensor(asb[:], at[:], recb[:],
                                            OP.mult)
                    attn_sb[h] = ("asb", asb)

                def oproj(st):
                    o = opool.tile([128, H], BF16, tag="osb")
                    for oc in range(4):
                        op = ao_ps.tile([128, 512], F32, tag="ao")
                        for ha in range(2):
                            for h in range(HPC):
                                nc.tensor.matmul(
                                    op[:, ha * 256:(ha + 1) * 256],
                                    attn_sb[h][1][:,
                                                  st * 128:(st + 1) * 128],
                                    ow_sb[:, h, oc * 512 + ha * 256:
                                          oc * 512 + (ha + 1) * 256],
                                    start=(h == 0 and ha == 0),
                                    stop=(h == HPC - 1 and ha == 1))
                        dst = o[:, oc * 512:(oc + 1) * 512]
                        if (st + oc) % 2 == 0:
                            nc.vector.tensor_copy(dst, op[:])
                        else:
                            nc.scalar.activation(dst, op[:], AF.Copy)
                    row = (4 * p + st) * 128
                    nc.sync.dma_start(out_d.ap()[row:row + 128, :], o[:])

                yield lambda: head_sum(0)
                yield lambda: head_sum(1)
                yield lambda: head_av(0)
                yield lambda: head_av(1)
                yield lambda: oproj(0)
                yield lambda: oproj(1)
                yield lambda: oproj(2)
                yield lambda: oproj(3)

            def emit_quad(p_scores, p_tail):
                pairs = pair_list(p_scores) if p_scores is not None else []
                tgs = tail_groups(p_tail) if p_tail is not None else iter(())
                for h in range(HPC):
                    for (t0, t1) in pairs:
                        emit_score_pair(p_scores, h, t0, t1)
                        nxt = next(tgs, None)
                        if nxt is not None:
                            nxt()
                for nxt in tgs:
                    nxt()

            emit_quad(0, None)
            for p in range(1, NQUAD):
                emit_quad(p, p - 1)
            emit_quad(None, NQUAD - 1)


def _host_prep(x, cos, sin, norm_weight, qkv_w, o_w):
    import ml_dtypes
    bf = ml_dtypes.bfloat16

    x2 = np.ascontiguousarray(x.reshape(S, H).astype(np.float32))
    xT = np.ascontiguousarray(x2.T).astype(bf)                  # [H, S]
    xn = x2.astype(bf)                                          # [S, H]
    identf = np.eye(128, dtype=np.float32)

    wq = (qkv_w.astype(np.float32) * norm_weight.astype(np.float32)[None, :])

    cosext = np.ones((128, S), dtype=np.float32)
    cosext[:ROPE_N, :] = cos.astype(np.float32).T[:ROPE_N, :]
    cosext = cosext.astype(bf)
    sinext = np.ascontiguousarray(
        sin.astype(np.float32).T[:ROPE_N, :]).astype(bf)

    # smat[k, m]: out[m] = -tsin[m+16] (m<16), +tsin[m-16] (16<=m<32)
    smat = np.zeros((ROPE_N, ROPE_N), dtype=np.float32)
    for m in range(16):
        smat[m + 16, m] = -1.0
        smat[m, m + 16] = 1.0
    smat = smat.astype(bf)

    ones = np.ones((128, 1), dtype=bf)
    onef = np.ones((1, 1), dtype=np.float32)
    kk = np.arange(128)[:, None]
    qq = np.arange(128)[None, :]
    causalT = (kk <= qq).astype(np.float32)
    antiT = (kk >= qq).astype(np.float32)

    tri = np.stack([causalT, antiT], axis=1).astype(bf)   # [128, 2, 128]

    shared = dict(xT=xT, xn=xn, identf=identf, cosext=cosext,
                  sinext=sinext, smat=smat, ones=ones, onef=onef, tri=tri)

    in_maps = []
    for c in range(NCORES):
        h0 = HPC * c
        rows = []
        for h in (h0, h0 + 1):
            rows.append(wq[h * HD:(h + 1) * HD])             # Q_h
            rows.append(wq[H + h * HD:H + (h + 1) * HD])     # K_h
        w_qk = np.concatenate(rows, axis=0)                  # [512, 2048]
        wqkT = np.ascontiguousarray(
            w_qk.T.reshape(HT, 128, 4, 128).transpose(2, 1, 0, 3)).astype(bf)
        w_v = np.concatenate(
            [wq[2 * H + h * HD:2 * H + (h + 1) * HD] for h in (h0, h0 + 1)],
            axis=0)                                          # [256, 2048]
        wvT = np.ascontiguousarray(
            w_v.T.reshape(HT, 128, HPC * 128).transpose(1, 0, 2)).astype(bf)
        ow_cols = np.concatenate(
            [o_w.astype(np.float32)[:, h * HD:(h + 1) * HD]
             for h in (h0, h0 + 1)], axis=1)                 # [2048, 256]
        owT = np.ascontiguousarray(
            ow_cols.T.reshape(HPC, 128, H).transpose(1, 0, 2)).astype(bf)
        m = dict(shared)
        m["wqk"] = wqkT
        m["wv"] = wvT
        m["ow"] = owT
        in_maps.append(m)
    return in_maps


def kernel(x, cos, sin, norm_weight, qkv_w, o_w, _trace=False, _tmpdir=None):
    x = np.asarray(x); cos = np.asarray(cos); sin = np.asarray(sin)
    norm_weight = np.asarray(norm_weight)
    qkv_w = np.asarray(qkv_w); o_w = np.asarray(o_w)

    if "nc" not in _CACHED:
        _CACHED["nc"] = _build_program()
    nc = _CACHED["nc"]

    in_maps = _host_prep(x, cos, sin, norm_weight, qkv_w, o_w)
    if _trace:
        _install_ntff_hook()
    res = bass_utils.run_bass_kernel_spmd(
        nc, in_maps, core_ids=list(range(NCORES)),
        trace=_trace, tmpdir=_tmpdir)

    out = np.zeros((S, H), dtype=np.float32)
    for c in range(NCORES):
        out += res.results[c]["out"].astype(np.float32)
    result = out.reshape(B, S, H)
    if _trace:
        return result, res
    return result
